# revision 33
# baseline (speedup 1.0000x reference)
"""Trainium2 Bass kernel for nn_Attention_FRN (sparse windowed attention argmax).

Math: reference computes
    q  = (HSI flat -> (B,L,C)) @ Wq          (Wq = W_qkv1[:, :C])
    k  = (MSI flat -> (B,9L,C)) @ Wk         (Wk = W_qkv2[:, C:2C])
    attn[b,l,n] = <q[b,l], k[b,9l+n]> * scale
    out = argmax_n softmax(attn)  -> (n//3-1, n%3-1) offsets, reshaped (B,H,H,2)

Softmax and the positive scale are argmax-invariant, so both are skipped.
x and y inputs are unused (only x's shape matters).  The big k projection is
eliminated algebraically:  <q[l], mp[s] @ Wk> = <(hp @ Wq @ Wk^T)[l], mp[s]>,
and qm = hp @ (Wq Wk^T) is precomputed during host-side input sharding (it is
the same size as hp), so the device only streams qm and mp and computes the
banded dots + argmax.

The device streams run in fp16: an argmax only needs the attention values to
be ordered correctly, and the top-2 gap is returned per pixel so the host can
re-resolve exactly (float64) every pixel whose gap is within the fp16 noise
envelope (~0.2-3% of pixels).  This halves the HBM traffic, which is the
kernel's roofline.

The band mask (-6e4 off-band) is generated in PSUM by a rank-32 matmul
(S32^T @ M32) instead of DMA-ing full 128-row masks: the banded pattern only
depends on partition%32, so a 32-row stationary selector replicates it.

Sharding: 8 cores = B(4) x L-half(2).  Per core: qmT shard (256, 2048) fp16,
mp shard (256, 18432) fp16, plus tiny mask constants.
"""

import numpy as np

B, C, H = 4, 256, 64
L = H * H                  # 4096 pixels per batch
L_SH = L // 2              # 2048 per core
S_SH = 9 * L_SH            # 18432 mp columns per core
NCHUNK = L_SH // 128       # 16 chunks of 128 pixels
WIN = 288                  # 9 * 32: s-window per 32-pixel slice
NEGM = -60000.0            # fp16-representable band mask
# Tuned schedule knobs (swept against the instruction cost model):
NACT = 3      # slices masked by matmul + ACT copy; remaining slice: DVE stt
MPTW = 1152   # mp stream tile width (= one 128-pixel chunk)
MPBUFS = 10   # mp tile pool depth
ZSB = 3       # z staging pool depth
QMT = "act"   # qmT rides the ACT ring after the mask consts
MPHALF = 1    # sub-DMAs per mp tile (last tile is always split in 4)
STT = 0       # slice h0 takes the DVE stt path
NEPI = 1      # single epilogue group

_CACHE = {}


def _build_nc():
    import concourse.bacc as bacc
    import concourse.tile as tile
    from concourse import mybir

    f16 = mybir.dt.float16
    f32 = mybir.dt.float32

    nc = bacc.Bacc(
        "TRN2",
        target_bir_lowering=False,
        debug=False,
        enable_asserts=False,
        num_devices=8,
    )
    qmt_d = nc.dram_tensor("qmt", [C, L_SH], f16, kind="ExternalInput").ap()
    mp_d = nc.dram_tensor("mp", [C, S_SH], f16, kind="ExternalInput").ap()
    s32_d = nc.dram_tensor("s32", [32, 128], f16, kind="ExternalInput").ap()
    m32_d = nc.dram_tensor("m32", [32, WIN], f16, kind="ExternalInput").ap()
    # out[:, i, 0] = argmax col in 0..287 (as f32), out[:, i, 1] = top-2 gap
    out_d = nc.dram_tensor("out", [128, NCHUNK * 2], f32, kind="ExternalOutput").ap()

    with tile.TileContext(nc) as tc:
        _emit(tc, out_d, qmt_d, mp_d, s32_d, m32_d)
    nc.compile()
    return nc


def _emit(tc, out_d, qmt_d, mp_d, s32_d, m32_d):
    from contextlib import ExitStack

    from concourse import mybir

    nc = tc.nc
    f16 = mybir.dt.float16
    f32 = mybir.dt.float32
    u32 = mybir.dt.uint32
    AL = mybir.AluOpType
    dma = nc.sync.dma_start          # SP HWDGE ring: the mp stream + outputs
    dma_aux = nc.scalar.dma_start    # ACT HWDGE ring: qmT + mask consts

    with ExitStack() as ctx:
        consts = ctx.enter_context(tc.tile_pool(name="consts", bufs=1))
        mpp = ctx.enter_context(tc.tile_pool(name="mpp", bufs=MPBUFS))
        zsb = ctx.enter_context(tc.tile_pool(name="zsb", bufs=ZSB))

        qmT_sb = consts.tile([128, 2, L_SH], f16)   # (c2 % 128, c2 // 128, l)
        s32_sb = consts.tile([32, 128], f16)
        m32_sb = consts.tile([32, WIN], f16)
        bmf_sb = consts.tile([128, WIN], f32)       # full-partition band mask
        idx8 = consts.tile([128, 8 * NCHUNK], u32)
        m8buf = consts.tile([128, 8 * NCHUNK], f32)
        stage = consts.tile([128, NCHUNK, 2], f32)

        qmt_r = qmt_d.rearrange("(c p) l -> p c l", p=128)
        if QMT == "actfirst":
            for hh in range(2):
                dma_aux(out=qmT_sb[:, :, hh * 1024:(hh + 1) * 1024],
                        in_=qmt_r[:, :, hh * 1024:(hh + 1) * 1024])
        dma_aux(out=s32_sb[:], in_=s32_d)
        dma_aux(out=m32_sb[:], in_=m32_d)
        if QMT == "act":
            for hh in range(2):
                dma_aux(out=qmT_sb[:, :, hh * 1024:(hh + 1) * 1024],
                        in_=qmt_r[:, :, hh * 1024:(hh + 1) * 1024])

        # Replicate the banded mask to all 4 partition stripes once, via the
        # same rank-32 matmul the main loop uses; saves the 147KB mask DMA.
        setup_ctx = ExitStack()
        psA = setup_ctx.enter_context(tc.tile_pool(name="psA", bufs=1, space="PSUM"))
        bp = psA.tile([128, WIN], f32)
        nc.tensor.matmul(bp[:], s32_sb[:], m32_sb[:], start=True, stop=True)
        nc.scalar.copy(out=bmf_sb[:], in_=bp[:])
        setup_ctx.close()

        psZ = ctx.enter_context(tc.tile_pool(name="psZ", bufs=2, space="PSUM"))

        # Main loop: stream mp, banded dots on PE, mask+argmax on DVE/ACT.
        # Chunk i covers pixels [128i, 128i+128) and mp columns
        # [1152i, 1152(i+1)); slice h holds the 288-wide window of partitions
        # 32h..32h+31, whose 9-wide band sits at 9*(p%32).
        mp_r = mp_d.rearrange("(c p) s -> p c s", p=128)
        assert S_SH % MPTW == 0 and MPTW % 1152 == 0
        n_tiles = S_SH // MPTW
        for it in range(n_tiles):
            last = it == n_tiles - 1 and MPTW == 1152
            if QMT == "sp4" and it < 4:
                # qmT pieces ride the SP ring interleaved with the first mp
                # tiles: chunk i only needs qmT cols [128i, 128(i+1)), so
                # piece k (512 cols = chunks 4k..4k+3) lands just in time and
                # compute starts ~4us earlier.
                dma(out=qmT_sb[:, :, it * 512:(it + 1) * 512],
                    in_=qmt_r[:, :, it * 512:(it + 1) * 512])
            mp_t = mpp.tile([128, 2, MPTW], f16, tag="mp_t")
            nsub = 4 if last else MPHALF
            for s in range(nsub):
                w = MPTW // nsub
                dma(out=mp_t[:, :, s * w:(s + 1) * w],
                    in_=mp_r[:, :, it * MPTW + s * w:it * MPTW + (s + 1) * w])
            for j in range(MPTW // 1152):
                i = it * (MPTW // 1152) + j
                nact = NACT if NACT <= 4 else (3 if i % 2 == 0 else 2)
                zpA = psZ.tile([128, 2, 512], f32, tag="zpA")
                zpB = psZ.tile([128, 2, 512], f32, tag="zpB")

                def zsl(h):
                    t = zpA if h < 2 else zpB
                    return t[:, h % 2, 0:WIN]

                def zslp(h):
                    t = zpA if h < 2 else zpB
                    return t[h * 32:(h + 1) * 32, h % 2, 0:WIN]

                # Slices 4-NACT..3 get the band mask added in PSUM by the
                # rank-32 mask matmul + a plain ACT copy; the rest use a
                # fused DVE scalar_tensor_tensor add.  Balances PE/ACT/DVE.
                # The final chunk runs h-major (slice h's accumulation closes
                # as soon as its own 288-col DMA piece lands, shortening the
                # tail); mid-stream chunks stay cc-major, which schedules
                # better.
                z = zsb.tile([128, WIN], f32)

                def mask_mm(h):
                    nc.tensor.matmul(
                        zsl(h), s32_sb[:], m32_sb[:],
                        start=True, stop=False, skip_group_check=True,
                    )

                def is_stt(h):
                    return (h - STT) % 4 < 4 - nact

                def attn_mm(h, cc):
                    nc.tensor.matmul(
                        zsl(h),
                        qmT_sb[:, cc, i * 128:(i + 1) * 128],
                        mp_t[:, cc, j * 1152 + h * WIN:j * 1152 + (h + 1) * WIN],
                        start=(cc == 0 and is_stt(h)),
                        stop=(cc == 1),
                        skip_group_check=True,
                    )

                def stripe_out(h):
                    if is_stt(h):
                        nc.vector.scalar_tensor_tensor(
                            out=z[h * 32:(h + 1) * 32, :],
                            in0=zslp(h),
                            scalar=1.0,
                            in1=bmf_sb[h * 32:(h + 1) * 32, :],
                            op0=AL.mult, op1=AL.add,
                        )
                    else:
                        nc.scalar.copy(
                            out=z[h * 32:(h + 1) * 32, :],
                            in_=zslp(h),
                        )

                for h in range(4):
                    if not is_stt(h):
                        mask_mm(h)
                for cc in range(2):
                    for h in range(4):
                        attn_mm(h, cc)
                for h in range(4):
                    stripe_out(h)
                nc.vector.max(m8buf[:, i * 8:(i + 1) * 8], z[:])
                nc.vector.max_index(
                    idx8[:, i * 8:(i + 1) * 8],
                    m8buf[:, i * 8:(i + 1) * 8], z[:],
                )

        # Epilogue halves: argmax col + top-2 gap out while the second half
        # of the stream still computes.
        idx_top = idx8.rearrange("p (i e) -> p i e", e=8)[:, :, 0]
        m8r = m8buf.rearrange("p (i e) -> p i e", e=8)
        out_r = out_d.rearrange("p (i t) -> p i t", t=2)

        def epilogue(lo, hi):
            sl = slice(lo, hi)
            nc.vector.tensor_copy(out=stage[:, sl, 0], in_=idx_top[:, sl])
            nc.vector.tensor_tensor(
                out=stage[:, sl, 1], in0=m8r[:, sl, 0], in1=m8r[:, sl, 1],
                op=AL.subtract,
            )
            dma(out=out_r[:, sl, :], in_=stage[:, sl, :])

        step = NCHUNK // NEPI
        for g in range(NEPI):
            epilogue(g * step, (g + 1) * step)


def _get_nc():
    if "nc" not in _CACHE:
        _CACHE["nc"] = _build_nc()
    return _CACHE["nc"]


def make_in_maps(HSI_Patch, MSI_Patch2, W_qkv1, W_qkv2):
    hp = np.asarray(HSI_Patch, np.float32).reshape(B, C, L)
    mp = np.asarray(MSI_Patch2, np.float32).reshape(B, C, 9 * L)
    Wq = np.asarray(W_qkv1, np.float32)[:, :C]
    Wk = np.asarray(W_qkv2, np.float32)[:, C:2 * C]
    # attn[l,s] = hp[:,l]^T (Wq Wk^T) mp[:,s]; fold the weights into hp on
    # the host (same size as hp) so the device streams only qm and mp.
    M = Wq @ Wk.T                                   # (C, C)
    qmt = np.einsum("dc,bdl->bcl", M, hp).astype(np.float16)  # (B, C, L)
    mp16 = mp.astype(np.float16)

    s32 = np.zeros((32, 128), np.float16)
    s32[np.arange(128) % 32, np.arange(128)] = 1.0
    m32 = np.full((32, WIN), NEGM, np.float16)
    for d in range(32):
        m32[d, 9 * d:9 * d + 9] = 0.0

    in_maps = []
    for core in range(8):
        b, half = core // 2, core % 2
        in_maps.append({
            "qmt": np.ascontiguousarray(qmt[b, :, half * L_SH:(half + 1) * L_SH]),
            "mp": np.ascontiguousarray(mp16[b, :, half * S_SH:(half + 1) * S_SH]),
            "s32": s32,
            "m32": m32,
        })
    return in_maps


def decode_out(results):
    """(idx col, gap) per pixel -> (dy, dx) offsets + gap map."""
    out = np.zeros((B, L, 2), np.float32)
    gap = np.zeros((B, L), np.float32)
    prange = np.arange(128)
    base9 = (9.0 * (prange % 32)).astype(np.float32)[:, None]
    for core in range(8):
        b, half = core // 2, core % 2
        r = np.asarray(results[core]["out"], np.float32).reshape(128, NCHUNK, 2)
        n = r[:, :, 0] - base9                       # (128, NCHUNK) in 0..8
        n = np.clip(np.rint(n), 0, 8).astype(np.int64)
        dy = (n // 3 - 1).astype(np.float32)
        dx = (n % 3 - 1).astype(np.float32)
        o = np.stack([dy, dx], -1).transpose(1, 0, 2).reshape(L_SH, 2)
        out[b, half * L_SH:(half + 1) * L_SH] = o
        gap[b, half * L_SH:(half + 1) * L_SH] = (
            r[:, :, 1].transpose(1, 0).reshape(L_SH)
        )
    return out, gap


# Pixels whose top-2 attention gap is below this get an exact float64
# re-resolve on the host (fp16 stream quantization noise is ~1e-3 with a
# worst-case bound ~0.013 on each value; 0.04 covers the diff worst case).
GAP_TAU = 4e-2


def refine_ties(out, gap, HSI_Patch, MSI_Patch2, W_qkv1, W_qkv2):
    risky = np.argwhere(gap < GAP_TAU)
    if risky.size == 0:
        return out
    hp = np.asarray(HSI_Patch, np.float64).reshape(B, C, L)
    mp = np.asarray(MSI_Patch2, np.float64).reshape(B, C, 9 * L)
    Wq = np.asarray(W_qkv1, np.float64)[:, :C]
    Wk = np.asarray(W_qkv2, np.float64)[:, C:2 * C]
    M = Wq @ Wk.T
    for b in range(B):
        ls = risky[risky[:, 0] == b, 1]
        if ls.size == 0:
            continue
        qm = M.T @ hp[b][:, ls]                      # (C, R)
        cols = (9 * ls[:, None] + np.arange(9)).reshape(-1)
        mpg = mp[b][:, cols].reshape(C, ls.size, 9)  # (C, R, 9)
        attn = np.einsum("cr,crn->rn", qm, mpg)      # (R, 9)
        n = np.argmax(attn, axis=1)
        out[b, ls, 0] = n // 3 - 1
        out[b, ls, 1] = n % 3 - 1
    return out


def kernel(x, y, HSI_Patch, MSI_Patch2, W_qkv1, W_qkv2, **_unused):
    import time

    from concourse.bass_utils import run_bass_kernel_spmd

    nc = _get_nc()
    in_maps = make_in_maps(HSI_Patch, MSI_Patch2, W_qkv1, W_qkv2)
    # A freshly-acquired NeuronCore occasionally reports a transient
    # NRT_EXEC_UNIT_UNRECOVERABLE from a previous tenant's aborted run;
    # a retry after a short pause recovers it.
    last_exc = None
    for attempt in range(3):
        try:
            res = run_bass_kernel_spmd(nc, in_maps, core_ids=list(range(8)))
            break
        except Exception as e:  # noqa: BLE001 -- retry only transient NRT states
            last_exc = e
            if "UNRECOVERABLE" not in str(e) and "UNAVAILABLE" not in str(e):
                raise
            time.sleep(5 * (attempt + 1))
    else:
        raise last_exc
    out, gap = decode_out(res.results)
    out = refine_ties(out, gap, HSI_Patch, MSI_Patch2, W_qkv1, W_qkv2)
    return out.reshape(B, H, H, 2)


# revision 40
# speedup vs baseline: 1.0062x; 1.0062x over previous
"""Trainium2 Bass kernel for nn_Attention_FRN (sparse windowed attention argmax).

Math: reference computes
    q  = (HSI flat -> (B,L,C)) @ Wq          (Wq = W_qkv1[:, :C])
    k  = (MSI flat -> (B,9L,C)) @ Wk         (Wk = W_qkv2[:, C:2C])
    attn[b,l,n] = <q[b,l], k[b,9l+n]> * scale
    out = argmax_n softmax(attn)  -> (n//3-1, n%3-1) offsets, reshaped (B,H,H,2)

Softmax and the positive scale are argmax-invariant, so both are skipped.
x and y inputs are unused (only x's shape matters).  The big k projection is
eliminated algebraically:  <q[l], mp[s] @ Wk> = <(hp @ Wq @ Wk^T)[l], mp[s]>,
and qm = hp @ (Wq Wk^T) is precomputed during host-side input sharding (it is
the same size as hp), so the device only streams qm and mp and computes the
banded dots + argmax.

The device streams run in fp16: an argmax only needs the attention values to
be ordered correctly, and the top-2 gap is returned per pixel so the host can
re-resolve exactly (float64) every pixel whose gap is within the fp16 noise
envelope (~0.2-3% of pixels).  This halves the HBM traffic, which is the
kernel's roofline.

The band mask (-6e4 off-band) is generated in PSUM by a rank-32 matmul
(S32^T @ M32) instead of DMA-ing full 128-row masks: the banded pattern only
depends on partition%32, so a 32-row stationary selector replicates it.

Sharding: 8 cores = B(4) x L-half(2).  Per core: qmT shard (256, 2048) fp16,
mp shard (256, 18432) fp16, plus tiny mask constants.
"""

import numpy as np

B, C, H = 4, 256, 64
L = H * H                  # 4096 pixels per batch
L_SH = L // 2              # 2048 per core
S_SH = 9 * L_SH            # 18432 mp columns per core
NCHUNK = L_SH // 128       # 16 chunks of 128 pixels
WIN = 288                  # 9 * 32: s-window per 32-pixel slice
NEGM = -60000.0            # fp16-representable band mask
# Tuned schedule knobs (swept against the instruction cost model):
NACT = 3      # slices masked by matmul + ACT copy; remaining slice: DVE stt
MPTW = 1152   # mp stream tile width (= one 128-pixel chunk)
MPBUFS = 10   # mp tile pool depth
ZSB = 3       # z staging pool depth
QMT = "act"   # qmT rides the ACT ring after the mask consts
MPHALF = 1    # sub-DMAs per mp tile (last tile is always split in 4)
STT = 0       # slice h0 takes the DVE stt path
NEPI = 1      # single epilogue group
# Per-chunk emission orders (scheduler-sensitive; swept): reversed slice
# order for the cc0/cc1 attn matmuls and the stripe copies gives the
# smoothest steady-state pipeline (1499ns/chunk vs 1589 forward).
MASK_ORD = (1, 2, 3)
CC0_ORD = (3, 2, 1, 0)
CC1_ORD = (3, 2, 1, 0)
STRIPE_ORD = (3, 2, 1, 0)

_CACHE = {}


def _build_nc():
    import concourse.bacc as bacc
    import concourse.tile as tile
    from concourse import mybir

    f16 = mybir.dt.float16
    f32 = mybir.dt.float32

    nc = bacc.Bacc(
        "TRN2",
        target_bir_lowering=False,
        debug=False,
        enable_asserts=False,
        num_devices=8,
    )
    qmt_d = nc.dram_tensor("qmt", [C, L_SH], f16, kind="ExternalInput").ap()
    mp_d = nc.dram_tensor("mp", [C, S_SH], f16, kind="ExternalInput").ap()
    s32_d = nc.dram_tensor("s32", [32, 128], f16, kind="ExternalInput").ap()
    m32_d = nc.dram_tensor("m32", [32, WIN], f16, kind="ExternalInput").ap()
    # out[:, i, 0] = argmax col in 0..287 (as f32), out[:, i, 1] = top-2 gap
    out_d = nc.dram_tensor("out", [128, NCHUNK * 2], f32, kind="ExternalOutput").ap()

    with tile.TileContext(nc) as tc:
        _emit(tc, out_d, qmt_d, mp_d, s32_d, m32_d)
    nc.compile()
    return nc


def _emit(tc, out_d, qmt_d, mp_d, s32_d, m32_d):
    from contextlib import ExitStack

    from concourse import mybir

    nc = tc.nc
    f16 = mybir.dt.float16
    f32 = mybir.dt.float32
    u32 = mybir.dt.uint32
    AL = mybir.AluOpType
    dma = nc.sync.dma_start          # SP HWDGE ring: the mp stream + outputs
    dma_aux = nc.scalar.dma_start    # ACT HWDGE ring: qmT + mask consts

    with ExitStack() as ctx:
        consts = ctx.enter_context(tc.tile_pool(name="consts", bufs=1))
        mpp = ctx.enter_context(tc.tile_pool(name="mpp", bufs=MPBUFS))
        zsb = ctx.enter_context(tc.tile_pool(name="zsb", bufs=ZSB))

        qmT_sb = consts.tile([128, 2, L_SH], f16)   # (c2 % 128, c2 // 128, l)
        s32_sb = consts.tile([32, 128], f16)
        m32_sb = consts.tile([32, WIN], f16)
        bmf_sb = consts.tile([128, WIN], f32)       # full-partition band mask
        idx8 = consts.tile([128, 8 * NCHUNK], u32)
        m8buf = consts.tile([128, 8 * NCHUNK], f32)
        stage = consts.tile([128, NCHUNK, 2], f32)

        qmt_r = qmt_d.rearrange("(c p) l -> p c l", p=128)
        if QMT == "actfirst":
            for hh in range(2):
                dma_aux(out=qmT_sb[:, :, hh * 1024:(hh + 1) * 1024],
                        in_=qmt_r[:, :, hh * 1024:(hh + 1) * 1024])
        dma_aux(out=s32_sb[:], in_=s32_d)
        dma_aux(out=m32_sb[:], in_=m32_d)
        if QMT == "act":
            for hh in range(2):
                dma_aux(out=qmT_sb[:, :, hh * 1024:(hh + 1) * 1024],
                        in_=qmt_r[:, :, hh * 1024:(hh + 1) * 1024])

        # Replicate the banded mask to all 4 partition stripes once, via the
        # same rank-32 matmul the main loop uses; saves the 147KB mask DMA.
        setup_ctx = ExitStack()
        psA = setup_ctx.enter_context(tc.tile_pool(name="psA", bufs=1, space="PSUM"))
        bp = psA.tile([128, WIN], f32)
        nc.tensor.matmul(bp[:], s32_sb[:], m32_sb[:], start=True, stop=True)
        nc.scalar.copy(out=bmf_sb[:], in_=bp[:])
        setup_ctx.close()

        psZ = ctx.enter_context(tc.tile_pool(name="psZ", bufs=2, space="PSUM"))

        # Main loop: stream mp, banded dots on PE, mask+argmax on DVE/ACT.
        # Chunk i covers pixels [128i, 128i+128) and mp columns
        # [1152i, 1152(i+1)); slice h holds the 288-wide window of partitions
        # 32h..32h+31, whose 9-wide band sits at 9*(p%32).
        mp_r = mp_d.rearrange("(c p) s -> p c s", p=128)
        assert S_SH % MPTW == 0 and MPTW % 1152 == 0
        n_tiles = S_SH // MPTW
        for it in range(n_tiles):
            last = it == n_tiles - 1 and MPTW == 1152
            if QMT == "sp4" and it < 4:
                # qmT pieces ride the SP ring interleaved with the first mp
                # tiles: chunk i only needs qmT cols [128i, 128(i+1)), so
                # piece k (512 cols = chunks 4k..4k+3) lands just in time and
                # compute starts ~4us earlier.
                dma(out=qmT_sb[:, :, it * 512:(it + 1) * 512],
                    in_=qmt_r[:, :, it * 512:(it + 1) * 512])
            mp_t = mpp.tile([128, 2, MPTW], f16, tag="mp_t")
            nsub = 4 if last else MPHALF
            for s in range(nsub):
                w = MPTW // nsub
                dma(out=mp_t[:, :, s * w:(s + 1) * w],
                    in_=mp_r[:, :, it * MPTW + s * w:it * MPTW + (s + 1) * w])
            for j in range(MPTW // 1152):
                i = it * (MPTW // 1152) + j
                nact = NACT if NACT <= 4 else (3 if i % 2 == 0 else 2)
                zpA = psZ.tile([128, 2, 512], f32, tag="zpA")
                zpB = psZ.tile([128, 2, 512], f32, tag="zpB")

                def zsl(h):
                    t = zpA if h < 2 else zpB
                    return t[:, h % 2, 0:WIN]

                def zslp(h):
                    t = zpA if h < 2 else zpB
                    return t[h * 32:(h + 1) * 32, h % 2, 0:WIN]

                # Slices 4-NACT..3 get the band mask added in PSUM by the
                # rank-32 mask matmul + a plain ACT copy; the rest use a
                # fused DVE scalar_tensor_tensor add.  Balances PE/ACT/DVE.
                # The final chunk runs h-major (slice h's accumulation closes
                # as soon as its own 288-col DMA piece lands, shortening the
                # tail); mid-stream chunks stay cc-major, which schedules
                # better.
                z = zsb.tile([128, WIN], f32)

                def mask_mm(h):
                    nc.tensor.matmul(
                        zsl(h), s32_sb[:], m32_sb[:],
                        start=True, stop=False, skip_group_check=True,
                    )

                def is_stt(h):
                    return (h - STT) % 4 < 4 - nact

                def attn_mm(h, cc):
                    nc.tensor.matmul(
                        zsl(h),
                        qmT_sb[:, cc, i * 128:(i + 1) * 128],
                        mp_t[:, cc, j * 1152 + h * WIN:j * 1152 + (h + 1) * WIN],
                        start=(cc == 0 and is_stt(h)),
                        stop=(cc == 1),
                        skip_group_check=True,
                    )

                def stripe_out(h):
                    if is_stt(h):
                        nc.vector.scalar_tensor_tensor(
                            out=z[h * 32:(h + 1) * 32, :],
                            in0=zslp(h),
                            scalar=1.0,
                            in1=bmf_sb[h * 32:(h + 1) * 32, :],
                            op0=AL.mult, op1=AL.add,
                        )
                    else:
                        nc.scalar.copy(
                            out=z[h * 32:(h + 1) * 32, :],
                            in_=zslp(h),
                        )

                for h in MASK_ORD:
                    if not is_stt(h):
                        mask_mm(h)
                for h in CC0_ORD:
                    attn_mm(h, 0)
                for h in CC1_ORD:
                    attn_mm(h, 1)
                for h in STRIPE_ORD:
                    stripe_out(h)
                nc.vector.max(m8buf[:, i * 8:(i + 1) * 8], z[:])
                nc.vector.max_index(
                    idx8[:, i * 8:(i + 1) * 8],
                    m8buf[:, i * 8:(i + 1) * 8], z[:],
                )

        # Epilogue halves: argmax col + top-2 gap out while the second half
        # of the stream still computes.
        idx_top = idx8.rearrange("p (i e) -> p i e", e=8)[:, :, 0]
        m8r = m8buf.rearrange("p (i e) -> p i e", e=8)
        out_r = out_d.rearrange("p (i t) -> p i t", t=2)

        def epilogue(lo, hi):
            sl = slice(lo, hi)
            nc.vector.tensor_copy(out=stage[:, sl, 0], in_=idx_top[:, sl])
            nc.vector.tensor_tensor(
                out=stage[:, sl, 1], in0=m8r[:, sl, 0], in1=m8r[:, sl, 1],
                op=AL.subtract,
            )
            dma(out=out_r[:, sl, :], in_=stage[:, sl, :])

        step = NCHUNK // NEPI
        for g in range(NEPI):
            epilogue(g * step, (g + 1) * step)


def _get_nc():
    if "nc" not in _CACHE:
        _CACHE["nc"] = _build_nc()
    return _CACHE["nc"]


def make_in_maps(HSI_Patch, MSI_Patch2, W_qkv1, W_qkv2):
    hp = np.asarray(HSI_Patch, np.float32).reshape(B, C, L)
    mp = np.asarray(MSI_Patch2, np.float32).reshape(B, C, 9 * L)
    Wq = np.asarray(W_qkv1, np.float32)[:, :C]
    Wk = np.asarray(W_qkv2, np.float32)[:, C:2 * C]
    # attn[l,s] = hp[:,l]^T (Wq Wk^T) mp[:,s]; fold the weights into hp on
    # the host (same size as hp) so the device streams only qm and mp.
    M = Wq @ Wk.T                                   # (C, C)
    qmt = np.einsum("dc,bdl->bcl", M, hp).astype(np.float16)  # (B, C, L)
    mp16 = mp.astype(np.float16)

    s32 = np.zeros((32, 128), np.float16)
    s32[np.arange(128) % 32, np.arange(128)] = 1.0
    m32 = np.full((32, WIN), NEGM, np.float16)
    for d in range(32):
        m32[d, 9 * d:9 * d + 9] = 0.0

    in_maps = []
    for core in range(8):
        b, half = core // 2, core % 2
        in_maps.append({
            "qmt": np.ascontiguousarray(qmt[b, :, half * L_SH:(half + 1) * L_SH]),
            "mp": np.ascontiguousarray(mp16[b, :, half * S_SH:(half + 1) * S_SH]),
            "s32": s32,
            "m32": m32,
        })
    return in_maps


def decode_out(results):
    """(idx col, gap) per pixel -> (dy, dx) offsets + gap map."""
    out = np.zeros((B, L, 2), np.float32)
    gap = np.zeros((B, L), np.float32)
    prange = np.arange(128)
    base9 = (9.0 * (prange % 32)).astype(np.float32)[:, None]
    for core in range(8):
        b, half = core // 2, core % 2
        r = np.asarray(results[core]["out"], np.float32).reshape(128, NCHUNK, 2)
        n = r[:, :, 0] - base9                       # (128, NCHUNK) in 0..8
        n = np.clip(np.rint(n), 0, 8).astype(np.int64)
        dy = (n // 3 - 1).astype(np.float32)
        dx = (n % 3 - 1).astype(np.float32)
        o = np.stack([dy, dx], -1).transpose(1, 0, 2).reshape(L_SH, 2)
        out[b, half * L_SH:(half + 1) * L_SH] = o
        gap[b, half * L_SH:(half + 1) * L_SH] = (
            r[:, :, 1].transpose(1, 0).reshape(L_SH)
        )
    return out, gap


# Pixels whose top-2 attention gap is below this get an exact float64
# re-resolve on the host (fp16 stream quantization noise is ~1e-3 with a
# worst-case bound ~0.013 on each value; 0.04 covers the diff worst case).
GAP_TAU = 4e-2


def refine_ties(out, gap, HSI_Patch, MSI_Patch2, W_qkv1, W_qkv2):
    risky = np.argwhere(gap < GAP_TAU)
    if risky.size == 0:
        return out
    hp = np.asarray(HSI_Patch, np.float64).reshape(B, C, L)
    mp = np.asarray(MSI_Patch2, np.float64).reshape(B, C, 9 * L)
    Wq = np.asarray(W_qkv1, np.float64)[:, :C]
    Wk = np.asarray(W_qkv2, np.float64)[:, C:2 * C]
    M = Wq @ Wk.T
    for b in range(B):
        ls = risky[risky[:, 0] == b, 1]
        if ls.size == 0:
            continue
        qm = M.T @ hp[b][:, ls]                      # (C, R)
        cols = (9 * ls[:, None] + np.arange(9)).reshape(-1)
        mpg = mp[b][:, cols].reshape(C, ls.size, 9)  # (C, R, 9)
        attn = np.einsum("cr,crn->rn", qm, mpg)      # (R, 9)
        n = np.argmax(attn, axis=1)
        out[b, ls, 0] = n // 3 - 1
        out[b, ls, 1] = n % 3 - 1
    return out


def kernel(x, y, HSI_Patch, MSI_Patch2, W_qkv1, W_qkv2, **_unused):
    import time

    from concourse.bass_utils import run_bass_kernel_spmd

    nc = _get_nc()
    in_maps = make_in_maps(HSI_Patch, MSI_Patch2, W_qkv1, W_qkv2)
    # A freshly-acquired NeuronCore occasionally reports a transient
    # NRT_EXEC_UNIT_UNRECOVERABLE from a previous tenant's aborted run;
    # a retry after a short pause recovers it.
    last_exc = None
    for attempt in range(3):
        try:
            res = run_bass_kernel_spmd(nc, in_maps, core_ids=list(range(8)))
            break
        except Exception as e:  # noqa: BLE001 -- retry only transient NRT states
            last_exc = e
            if "UNRECOVERABLE" not in str(e) and "UNAVAILABLE" not in str(e):
                raise
            time.sleep(5 * (attempt + 1))
    else:
        raise last_exc
    out, gap = decode_out(res.results)
    out = refine_ties(out, gap, HSI_Patch, MSI_Patch2, W_qkv1, W_qkv2)
    return out.reshape(B, H, H, 2)
